# revision 16
# baseline (speedup 1.0000x reference)
"""BitNet-style row-parallel linear on 8 TRN2 NeuronCores.

Reference computes: out[b,s,o] = sum_d x[b,s,d] * sign(w[o,d]) + bias[o]
  x: [4, 2048, 4096] f32, w: [4096, 4096] f32, bias: [4096] f32.

Strategy: data-parallel over the 8192 (b*s) rows — each of the 8 cores
computes a 1024-row slice of the output against the full binarized
weight. No collective needed; shards concatenate to the full output.
(The row-parallel/all-reduce hint costs a 128MB all-reduce per core;
sharding M instead makes the partial outputs disjoint.)

TensorE consumes both operands K-major, so the host preps:
  kxm = x_shard.T           [K=4096, M=1024]  (per core)
  kxn = sign(w).T           [K=4096, N=4096]  (same on every core)
Matmul runs in float32r (fp22 multiply, fp32 accumulate) — 4x faster
than true fp32 on the PE and far more accurate than bf16 inputs.
"""

import numpy as np

B, S, D_IN, D_OUT = 4, 2048, 4096, 4096
NCORES = 8
M_TOTAL = B * S
M_CORE = M_TOTAL // NCORES

import os

_cache = {}

# "f32r" (fp22 multiply, highest precision) or "bf16" (half the DMA
# traffic + fast weight load; weights are exactly representable).
DTYPE = os.environ.get("BK_DTYPE", "bf16")


IMPL = os.environ.get("BK_IMPL", "custom")


def _custom_body(nc, tc, kxm, kxn, out, mm_dt, mybir):
    """x^T stays SBUF-resident; sign(w)^T streams through once.

    Per n-block of 512 output columns, accumulate k into PSUM banks.
    Block 0 sweeps all 8 banks per k-tile (x still streaming in);
    later blocks run one bank at a time so evictions pipeline and the
    tail after the last matmul is a single evict+store.
    """
    P = 128
    KT = D_IN // P          # 32 k tiles
    MT = M_CORE // P        # 8 m tiles
    NW = 512
    NB = D_OUT // NW        # 8 n blocks
    f32 = mybir.dt.float32

    from contextlib import ExitStack
    with ExitStack() as ctx:
        kxm_pool = ctx.enter_context(tc.tile_pool(name="kxm", bufs=1))
        kxn_pool = ctx.enter_context(tc.tile_pool(name="kxn", bufs=9))
        psum_pool = ctx.enter_context(
            tc.tile_pool(name="psum", bufs=8, space="PSUM"))
        out_pool = ctx.enter_context(tc.tile_pool(name="outp", bufs=8))

        def issue_chunk(nb, c, k0, sz):
            # one kxn chunk: k tiles [k0, k0+sz) of n block nb
            t = kxn_pool.tile([P, sz, NW], mm_dt, tag="kxn",
                              name=f"kxn_{nb}_{c}", bufs=20)
            src = kxn[k0 * P:(k0 + sz) * P, nb * NW:(nb + 1) * NW]
            nc.sync.dma_start(
                out=t, in_=src.rearrange("(ko ki) n -> ki ko n", ki=P))
            return [t[:, i, :] for i in range(sz)]

        def issue_chunks(nb, sizes):
            rhs, k0 = [], 0
            for c, sz in enumerate(sizes):
                rhs += issue_chunk(nb, c, k0, sz)
                k0 += sz
            return rhs

        kxm_tiles = []

        def issue_kxm(k):
            kt = kxm_pool.tile([P, M_CORE], mm_dt, tag="kxm",
                               name=f"kxm_{k}", bufs=KT)
            eng = nc.scalar if k % 2 == 0 else nc.gpsimd
            eng.dma_start(out=kt[:, :], in_=kxm[k * P:(k + 1) * P, :])
            kxm_tiles.append(kt)

        # Prologue interleave: tiny leading weight chunks so the first
        # matmul fires as early as possible, x tiles paced at the
        # k-loop rate on the scalar/gpsimd queues.
        sizes0 = [1, 1, 1, 1, 4, 4, 4, 4, 4, 4, 4]
        rhs0, k0 = [], 0
        issue_kxm(0)
        for c, sz in enumerate(sizes0):
            rhs0 += issue_chunk(0, c, k0, sz)
            k0 += sz
            for k in range(len(kxm_tiles), min(k0 + 2, KT)):
                issue_kxm(k)
        for k in range(len(kxm_tiles), KT):
            issue_kxm(k)

        next_rhs = rhs0
        for nb in range(NB):
            ncols = slice(nb * NW, (nb + 1) * NW)
            rhs_k = next_rhs
            psums = [psum_pool.tile([P, NW], f32, tag="ps", name=f"ps_{nb}_{i}")
                     for i in range(MT)]
            groups = [range(MT)] if nb == 0 else [[m] for m in range(MT)]
            for gi, ms in enumerate(groups):
                for k in range(KT):
                    for m in ms:
                        nc.tensor.matmul(
                            psums[m][:, :],
                            lhsT=kxm_tiles[k][:, m * P:(m + 1) * P],
                            rhs=rhs_k[k],
                            start=(k == 0), stop=(k == KT - 1))
                if gi == 0 and nb + 1 < NB:
                    next_rhs = issue_chunks(nb + 1, [4] * 8)
                for m in ms:
                    ot = out_pool.tile([P, NW], f32, tag="ot", name=f"ot_{nb}_{m}")
                    nc.vector.tensor_copy(out=ot[:, :], in_=psums[m][:, :])
                    nc.gpsimd.dma_start(
                        out=out[m * P:(m + 1) * P, ncols], in_=ot[:, :])


def _build():
    """Build + compile the 8-core SPMD Bass program once per process."""
    if "nc" in _cache:
        return _cache["nc"]

    import concourse.bacc as bacc
    import concourse.tile as tile
    import concourse.mybir as mybir
    from concourse.kernels.tile_matmul import matmul_tile_kernel

    mm_dt = {"f32r": mybir.dt.float32r, "bf16": mybir.dt.bfloat16}[DTYPE]

    nc = bacc.Bacc("TRN2", target_bir_lowering=False, debug=False,
                   enable_asserts=False, num_devices=NCORES)
    kxm = nc.dram_tensor("kxm", [D_IN, M_CORE], mm_dt,
                         kind="ExternalInput").ap()
    kxn = nc.dram_tensor("kxn", [D_IN, D_OUT], mm_dt,
                         kind="ExternalInput").ap()
    out = nc.dram_tensor("out", [M_CORE, D_OUT], mybir.dt.float32,
                         kind="ExternalOutput").ap()
    if IMPL == "custom":
        with tile.TileContext(nc) as tc:
            _custom_body(nc, tc, kxm, kxn, out, mm_dt, mybir)
    else:
        kw = {}
        if os.environ.get("BK_MAX_K_TILE"):
            kw["MAX_K_TILE_SIZE"] = int(os.environ["BK_MAX_K_TILE"])
        if os.environ.get("BK_SKIP_K_SNAKE"):
            kw["skip_k_snake"] = True
        if os.environ.get("BK_NO_CACHE_TILES"):
            kw["cache_tiles"] = False
        with tile.TileContext(nc) as tc:
            matmul_tile_kernel(tc, kxm, kxn, out, **kw)
    nc.compile()
    _cache["nc"] = nc
    return nc


def _prep_inputs(x, weight):
    if DTYPE == "bf16":
        import ml_dtypes
        np_dt = ml_dtypes.bfloat16
    else:
        np_dt = np.float32
    x2d = np.asarray(x, dtype=np.float32).reshape(M_TOTAL, D_IN)
    kxn = np.ascontiguousarray(np.sign(weight, dtype=np.float32).T.astype(np_dt))
    in_maps = []
    for c in range(NCORES):
        kxm = np.ascontiguousarray(x2d[c * M_CORE:(c + 1) * M_CORE].T.astype(np_dt))
        in_maps.append({"kxm": kxm, "kxn": kxn})
    return in_maps


def _run(x, weight, bias, trace=False):
    from concourse.bass_utils import run_bass_kernel_spmd

    nc = _build()
    in_maps = _prep_inputs(x, weight)
    res = run_bass_kernel_spmd(nc, in_maps, core_ids=list(range(NCORES)),
                               trace=trace)
    out = np.concatenate([res.results[c]["out"] for c in range(NCORES)],
                         axis=0)
    bias = np.asarray(bias, dtype=np.float32)
    if np.any(bias):
        out += bias
    return out.reshape(B, S, D_OUT), res


def kernel(x, weight, bias):
    out, _ = _run(x, weight, bias, trace=False)
    return out


# revision 17
# speedup vs baseline: 1.0109x; 1.0109x over previous
"""BitNet-style row-parallel linear on 8 TRN2 NeuronCores.

Reference computes: out[b,s,o] = sum_d x[b,s,d] * sign(w[o,d]) + bias[o]
  x: [4, 2048, 4096] f32, w: [4096, 4096] f32, bias: [4096] f32.

Strategy: data-parallel over the 8192 (b*s) rows — each of the 8 cores
computes a 1024-row slice of the output against the full binarized
weight. No collective needed; shards concatenate to the full output.
(The row-parallel/all-reduce hint costs a 128MB all-reduce per core;
sharding M instead makes the partial outputs disjoint.)

TensorE consumes both operands K-major, so the host preps:
  kxm = x_shard.T           [K=4096, M=1024]  (per core)
  kxn = sign(w).T           [K=4096, N=4096]  (same on every core)
Matmul runs in float32r (fp22 multiply, fp32 accumulate) — 4x faster
than true fp32 on the PE and far more accurate than bf16 inputs.
"""

import numpy as np

B, S, D_IN, D_OUT = 4, 2048, 4096, 4096
NCORES = 8
M_TOTAL = B * S
M_CORE = M_TOTAL // NCORES

import os

_cache = {}

# "f32r" (fp22 multiply, highest precision) or "bf16" (half the DMA
# traffic + fast weight load; weights are exactly representable).
DTYPE = os.environ.get("BK_DTYPE", "bf16")


IMPL = os.environ.get("BK_IMPL", "custom")


def _custom_body(nc, tc, kxm, kxn, out, mm_dt, mybir):
    """x^T stays SBUF-resident; sign(w)^T streams through once.

    Per n-block of 512 output columns, accumulate k into PSUM banks.
    Block 0 sweeps all 8 banks per k-tile (x still streaming in);
    later blocks run one bank at a time so evictions pipeline and the
    tail after the last matmul is a single evict+store.
    """
    P = 128
    KT = D_IN // P          # 32 k tiles
    MT = M_CORE // P        # 8 m tiles
    NW = 512
    NB = D_OUT // NW        # 8 n blocks
    f32 = mybir.dt.float32

    from contextlib import ExitStack
    with ExitStack() as ctx:
        kxm_pool = ctx.enter_context(tc.tile_pool(name="kxm", bufs=1))
        kxn_pool = ctx.enter_context(tc.tile_pool(name="kxn", bufs=9))
        psum_pool = ctx.enter_context(
            tc.tile_pool(name="psum", bufs=8, space="PSUM"))
        out_pool = ctx.enter_context(tc.tile_pool(name="outp", bufs=8))

        def issue_chunk(nb, c, k0, sz):
            # one kxn chunk: k tiles [k0, k0+sz) of n block nb
            t = kxn_pool.tile([P, sz, NW], mm_dt, tag="kxn",
                              name=f"kxn_{nb}_{c}", bufs=24)
            src = kxn[k0 * P:(k0 + sz) * P, nb * NW:(nb + 1) * NW]
            nc.sync.dma_start(
                out=t, in_=src.rearrange("(ko ki) n -> ki ko n", ki=P))
            return [t[:, i, :] for i in range(sz)]

        def issue_chunks(nb, sizes):
            rhs, k0 = [], 0
            for c, sz in enumerate(sizes):
                rhs += issue_chunk(nb, c, k0, sz)
                k0 += sz
            return rhs

        kxm_tiles = []

        def issue_kxm(k):
            kt = kxm_pool.tile([P, M_CORE], mm_dt, tag="kxm",
                               name=f"kxm_{k}", bufs=KT)
            eng = nc.scalar if k % 2 == 0 else nc.gpsimd
            eng.dma_start(out=kt[:, :], in_=kxm[k * P:(k + 1) * P, :])
            kxm_tiles.append(kt)

        # Prologue interleave: tiny leading weight chunks so the first
        # matmul fires as early as possible, x tiles paced at the
        # k-loop rate on the scalar/gpsimd queues.
        sizes0 = [2] * 16
        rhs0, k0 = [], 0
        issue_kxm(0)
        for c, sz in enumerate(sizes0):
            rhs0 += issue_chunk(0, c, k0, sz)
            k0 += sz
            for k in range(len(kxm_tiles), min(k0 + 2, KT)):
                issue_kxm(k)
        for k in range(len(kxm_tiles), KT):
            issue_kxm(k)

        next_rhs = rhs0
        for nb in range(NB):
            ncols = slice(nb * NW, (nb + 1) * NW)
            rhs_k = next_rhs
            psums = [psum_pool.tile([P, NW], f32, tag="ps", name=f"ps_{nb}_{i}")
                     for i in range(MT)]
            groups = [range(MT)] if nb == 0 else [[m] for m in range(MT)]
            for gi, ms in enumerate(groups):
                for k in range(KT):
                    for m in ms:
                        nc.tensor.matmul(
                            psums[m][:, :],
                            lhsT=kxm_tiles[k][:, m * P:(m + 1) * P],
                            rhs=rhs_k[k],
                            start=(k == 0), stop=(k == KT - 1))
                if gi == 0 and nb + 1 < NB:
                    next_rhs = issue_chunks(nb + 1, [4] * 8)
                for m in ms:
                    ot = out_pool.tile([P, NW], f32, tag="ot", name=f"ot_{nb}_{m}")
                    nc.vector.tensor_copy(out=ot[:, :], in_=psums[m][:, :])
                    nc.gpsimd.dma_start(
                        out=out[m * P:(m + 1) * P, ncols], in_=ot[:, :])


def _build():
    """Build + compile the 8-core SPMD Bass program once per process."""
    if "nc" in _cache:
        return _cache["nc"]

    import concourse.bacc as bacc
    import concourse.tile as tile
    import concourse.mybir as mybir
    from concourse.kernels.tile_matmul import matmul_tile_kernel

    mm_dt = {"f32r": mybir.dt.float32r, "bf16": mybir.dt.bfloat16}[DTYPE]

    nc = bacc.Bacc("TRN2", target_bir_lowering=False, debug=False,
                   enable_asserts=False, num_devices=NCORES)
    kxm = nc.dram_tensor("kxm", [D_IN, M_CORE], mm_dt,
                         kind="ExternalInput").ap()
    kxn = nc.dram_tensor("kxn", [D_IN, D_OUT], mm_dt,
                         kind="ExternalInput").ap()
    out = nc.dram_tensor("out", [M_CORE, D_OUT], mybir.dt.float32,
                         kind="ExternalOutput").ap()
    if IMPL == "custom":
        with tile.TileContext(nc) as tc:
            _custom_body(nc, tc, kxm, kxn, out, mm_dt, mybir)
    else:
        kw = {}
        if os.environ.get("BK_MAX_K_TILE"):
            kw["MAX_K_TILE_SIZE"] = int(os.environ["BK_MAX_K_TILE"])
        if os.environ.get("BK_SKIP_K_SNAKE"):
            kw["skip_k_snake"] = True
        if os.environ.get("BK_NO_CACHE_TILES"):
            kw["cache_tiles"] = False
        with tile.TileContext(nc) as tc:
            matmul_tile_kernel(tc, kxm, kxn, out, **kw)
    nc.compile()
    _cache["nc"] = nc
    return nc


def _prep_inputs(x, weight):
    if DTYPE == "bf16":
        import ml_dtypes
        np_dt = ml_dtypes.bfloat16
    else:
        np_dt = np.float32
    x2d = np.asarray(x, dtype=np.float32).reshape(M_TOTAL, D_IN)
    kxn = np.ascontiguousarray(np.sign(weight, dtype=np.float32).T.astype(np_dt))
    in_maps = []
    for c in range(NCORES):
        kxm = np.ascontiguousarray(x2d[c * M_CORE:(c + 1) * M_CORE].T.astype(np_dt))
        in_maps.append({"kxm": kxm, "kxn": kxn})
    return in_maps


def _run(x, weight, bias, trace=False):
    from concourse.bass_utils import run_bass_kernel_spmd

    nc = _build()
    in_maps = _prep_inputs(x, weight)
    res = run_bass_kernel_spmd(nc, in_maps, core_ids=list(range(NCORES)),
                               trace=trace)
    out = np.concatenate([res.results[c]["out"] for c in range(NCORES)],
                         axis=0)
    bias = np.asarray(bias, dtype=np.float32)
    if np.any(bias):
        out += bias
    return out.reshape(B, S, D_OUT), res


def kernel(x, weight, bias):
    out, _ = _run(x, weight, bias, trace=False)
    return out
